# revision 36
# baseline (speedup 1.0000x reference)
"""3x3 SAME conv (B=32, Cin=128, H=W=64, Cout=256) + bias + relu on 8 trn2 cores.

Strategy: data-parallel over batch (4 images per core, no collectives).
Per image, implicit GEMM: the input lives in SBUF as a zero-padded
[Cin=128, 66, 66] bf16 tile; for each of the 9 taps a [128cin x 128cout]
bf16 weight slice multiplies a shifted [128, 8rows*64cols] window,
accumulating fp32 in PSUM. bf16 operands enable the compiler's Fast
Weight Load path (4-XBUS LDWEIGHTS), so the per-matmul weight load hides
behind the previous matmul's 512-column stream and the PE runs at its
~213ns/matmul issue rate instead of ~242ns with f32r's 4-byte loads.
Bias+relu are fused on the scalar engine straight out of PSUM.

The host pre-pads the input and pre-casts input+weights to bf16 (RNE),
so the device does zero DVE work on the data path. Startup: the sync
HWDGE ring carries the need-ordered bulk data (chunk-0 weights, image
0's row bands, then images 1-3 whole — ring FIFO keeps the fat image
descriptors from starving the stream-start pieces), the scalar ring
stays light (bias, chunk-1 weights, and later the output stores), and
N=256 warmup matmuls on a memset tile bridge the PE from dispatch-ready
to data-ready so the HAM clock-gate busy window runs into the stream.
"""

from contextlib import ExitStack

import ml_dtypes
import numpy as np

import concourse.bass as bass
import concourse.tile as tile
from concourse import bacc, mybir
from concourse.bass_utils import run_bass_kernel_spmd

N_CORES = 8
B, C_IN, H, W = 32, 128, 64, 64
C_OUT, K = 256, 3
B_LOC = B // N_CORES          # images per core
N_CHUNK = C_OUT // 128        # cout chunks of 128
ROWS_PER_MM = 8               # 8 rows x 64 cols = 512 moving elements
N_RG = H // ROWS_PER_MM       # row groups per image
HP, WP = H + 2, W + 2         # padded

_COMPILED = None


def _build():
    nc = bacc.Bacc("TRN2", target_bir_lowering=False, debug=False,
                   num_devices=N_CORES)

    inp = nc.dram_tensor("inp", [B_LOC, C_IN, HP, WP], mybir.dt.bfloat16,
                         kind="ExternalInput").ap()
    # 1-column-shifted copy (padded cols 1:65): gives the kw=1 taps a
    # 4-byte-aligned, fully contiguous [128, 8x64] window — without it
    # they pay ~+7ns/matmul for 2-byte-misaligned SBUF reads.
    inp_s = nc.dram_tensor("inp_s", [B_LOC, C_IN, HP, W], mybir.dt.bfloat16,
                           kind="ExternalInput").ap()
    wt = nc.dram_tensor("wt", [N_CHUNK, C_IN, K * K, 128], mybir.dt.bfloat16,
                        kind="ExternalInput").ap()
    bias2 = nc.dram_tensor("bias2", [128, N_CHUNK], mybir.dt.float32,
                           kind="ExternalInput").ap()
    out = nc.dram_tensor("out", [B_LOC, C_OUT, H, W], mybir.dt.float32,
                         kind="ExternalOutput").ap()

    with tile.TileContext(nc) as tc, ExitStack() as ctx:
        consts = ctx.enter_context(tc.tile_pool(name="consts", bufs=1))
        pads = ctx.enter_context(tc.tile_pool(name="pads", bufs=1))
        outs = ctx.enter_context(tc.tile_pool(name="outs", bufs=6))
        psums = ctx.enter_context(tc.tile_pool(name="psums", bufs=6,
                                               space="PSUM"))
        wps = ctx.enter_context(tc.tile_pool(name="wps", bufs=1,
                                             space="PSUM"))

        # The two HWDGE rings pace everything by FIFO order — no SWDGE, no
        # timing hacks. Ring contents are ordered by need-by time, with the
        # stream-start gate (w chunk 0 + image-0 band 0) in front sharing
        # the early HBM window with nothing. Weights are chunk-major so
        # each cout-chunk is one contiguous 2304B-per-channel piece.
        w_r = consts.tile([128, N_CHUNK, K * K, 128], mybir.dt.bfloat16,
                          tag="w_r")
        b_sb = consts.tile([128, N_CHUNK], mybir.dt.float32, tag="b_sb")
        nc.sync.dma_start(out=w_r[:, 0], in_=wt[0])
        nc.scalar.dma_start(out=b_sb[:], in_=bias2[:])

        # Bridge the gap between PE dispatch-ready (~8us) and data-ready
        # (~10.9us) with N=256 matmuls on a memset tile: fine enough grain
        # that the last one ends within ~0.2us of the gate, keeping the HAM
        # clock-gate busy window continuous into the real stream.
        warm = consts.tile([128, 512], mybir.dt.bfloat16, tag="warm")
        nc.vector.memset(warm[:], 0.0)
        wpsum = wps.tile([128, ROWS_PER_MM * W], mybir.dt.float32,
                         tag="wpsum")
        for i in range(12):
            nc.tensor.matmul(wpsum[:, 0:256], warm[:, 0:128],
                             warm[:, 0:256], start=True, stop=True)

        pimgs = [pads.tile([128, HP, WP], mybir.dt.bfloat16,
                           name=f"pimg{i}", tag=f"pimg{i}")
                 for i in range(B_LOC)]
        pimgs_s = [pads.tile([128, HP, W], mybir.dt.bfloat16,
                             name=f"pimgs{i}", tag=f"pimgs{i}")
                   for i in range(B_LOC)]

        # One need-ordered data ring: the sync ring carries image 0's row
        # bands (rowgroup r reads padded rows 8r..8r+9) and then images 1-3
        # whole. Ring FIFO guarantees the bands are never starved by the
        # fat image descriptors (SDMA engines round-robin between rings
        # per PACKET, so a concurrent ring with 8712B descriptors would
        # take ~4x the bandwidth of the 2112B band pieces). The scalar
        # ring stays light — bias + chunk-1 weights — so the stores it
        # carries later never queue behind bulk loads.
        nc.scalar.dma_start(out=w_r[:, 1], in_=wt[1])
        bounds = [0, 10, 18, 34, 50, HP]
        for s in range(len(bounds) - 1):
            lo, hi = bounds[s], bounds[s + 1]
            nc.sync.dma_start(out=pimgs[0][:, lo:hi, :],
                              in_=inp[0, :, lo:hi, :])
            nc.sync.dma_start(out=pimgs_s[0][:, lo:hi, :],
                              in_=inp_s[0, :, lo:hi, :])
        for b in range(1, B_LOC):
            nc.sync.dma_start(out=pimgs[b][:], in_=inp[b])
            nc.sync.dma_start(out=pimgs_s[b][:], in_=inp_s[b])

        # Chunk-outer: all 8 rowgroups of chunk 0 (~15.5us of matmuls) run
        # before the first use of the chunk-1 weights, which land mid-sweep.
        for b in range(B_LOC):
            pimg = pimgs[b]
            for c in range(N_CHUNK):
                for r in range(N_RG):
                    acc = psums.tile([128, ROWS_PER_MM * W], mybir.dt.float32,
                                     tag="acc")
                    y0 = r * ROWS_PER_MM
                    for t in range(K * K):
                        kh, kw = divmod(t, K)
                        if kw == 1:
                            rhs = pimgs_s[b][:, y0 + kh:y0 + kh
                                             + ROWS_PER_MM, :]
                        else:
                            rhs = pimg[:, y0 + kh:y0 + kh + ROWS_PER_MM,
                                       kw:kw + W]
                        nc.tensor.matmul(acc[:],
                                         w_r[:, c, t, :],
                                         rhs,
                                         start=(t == 0), stop=(t == K * K - 1))
                    o = outs.tile([128, ROWS_PER_MM, W], mybir.dt.float32,
                                  tag="o")
                    acc_hw = acc[:].rearrange("p (h w) -> p h w",
                                              h=ROWS_PER_MM)
                    last = (b == B_LOC - 1 and r == N_RG - 1
                            and c == N_CHUNK - 1)
                    if last:
                        # Pipeline the exposed tail: activate + store in
                        # half-rowgroup pieces on both HWDGE rings, so the
                        # first store transfer overlaps the second half's
                        # activation instead of waiting for all of it.
                        h2 = ROWS_PER_MM // 2
                        nc.scalar.activation(o[:, 0:h2], acc_hw[:, 0:h2],
                                             mybir.ActivationFunctionType.Relu,
                                             bias=b_sb[:, c:c + 1], scale=1.0)
                        nc.sync.dma_start(
                            out=out[b, c * 128:(c + 1) * 128,
                                    y0:y0 + h2, :],
                            in_=o[:, 0:h2])
                        nc.scalar.activation(o[:, h2:ROWS_PER_MM],
                                             acc_hw[:, h2:ROWS_PER_MM],
                                             mybir.ActivationFunctionType.Relu,
                                             bias=b_sb[:, c:c + 1], scale=1.0)
                        nc.scalar.dma_start(
                            out=out[b, c * 128:(c + 1) * 128,
                                    y0 + h2:y0 + ROWS_PER_MM, :],
                            in_=o[:, h2:ROWS_PER_MM])
                    else:
                        nc.scalar.activation(o[:], acc_hw,
                                             mybir.ActivationFunctionType.Relu,
                                             bias=b_sb[:, c:c + 1], scale=1.0)
                        nc.scalar.dma_start(
                            out=out[b, c * 128:(c + 1) * 128,
                                    y0:y0 + ROWS_PER_MM, :],
                            in_=o[:])

    nc.compile()
    return nc


def _get_compiled():
    global _COMPILED
    if _COMPILED is None:
        _COMPILED = _build()
    return _COMPILED


def _run(inp, weight, bias, trace=False):
    inp = np.asarray(inp, dtype=np.float32)
    weight = np.asarray(weight, dtype=np.float32)
    bias = np.asarray(bias, dtype=np.float32)

    # Zero-pad to 66x66 and cast to bf16 host-side; also build the
    # 1-column-shifted copy (padded cols 1:65) for the kw=1 taps.
    inp_p = np.zeros((B, C_IN, HP, WP), dtype=np.float32)
    inp_p[:, :, 1:H + 1, 1:W + 1] = inp
    inp_p = inp_p.astype(ml_dtypes.bfloat16)
    inp_s = np.ascontiguousarray(inp_p[:, :, :, 1:W + 1])

    # weight [C_OUT, C_IN*K*K] -> [N_CHUNK, C_IN, K*K, 128] (chunk-major
    # lhsT layout: wt[c, ci, t, m] = weight[c*128+m, ci*9+t])
    wt = np.ascontiguousarray(
        weight.reshape(N_CHUNK, 128, C_IN, K * K).transpose(0, 2, 3, 1)
    ).astype(ml_dtypes.bfloat16)
    # bias [C_OUT] -> [128, N_CHUNK]: bias2[p, c] = bias[c*128 + p]
    bias2 = np.ascontiguousarray(bias.reshape(N_CHUNK, 128).T)

    nc = _get_compiled()
    in_maps = [
        {"inp": inp_p[i * B_LOC:(i + 1) * B_LOC],
         "inp_s": inp_s[i * B_LOC:(i + 1) * B_LOC],
         "wt": wt, "bias2": bias2}
        for i in range(N_CORES)
    ]
    res = run_bass_kernel_spmd(nc, in_maps, list(range(N_CORES)), trace=trace)
    full = np.concatenate([res.results[i]["out"] for i in range(N_CORES)],
                          axis=0)
    return full, res


def kernel(inp, weight, bias):
    full, _ = _run(inp, weight, bias, trace=False)
    return full


# revision 37
# speedup vs baseline: 1.0096x; 1.0096x over previous
"""3x3 SAME conv (B=32, Cin=128, H=W=64, Cout=256) + bias + relu on 8 trn2 cores.

Strategy: data-parallel over batch (4 images per core, no collectives).
Per image, implicit GEMM: the input lives in SBUF as a zero-padded
[Cin=128, 66, 66] bf16 tile; for each of the 9 taps a [128cin x 128cout]
bf16 weight slice multiplies a shifted [128, 8rows*64cols] window,
accumulating fp32 in PSUM. bf16 operands enable the compiler's Fast
Weight Load path (4-XBUS LDWEIGHTS), so the per-matmul weight load hides
behind the previous matmul's 512-column stream and the PE runs at its
~213ns/matmul issue rate instead of ~242ns with f32r's 4-byte loads.
Bias+relu are fused on the scalar engine straight out of PSUM.

The host pre-pads the input and pre-casts input+weights to bf16 (RNE),
so the device does zero DVE work on the data path. Startup: the sync
HWDGE ring carries the need-ordered bulk data (chunk-0 weights, image
0's row bands, then images 1-3 whole — ring FIFO keeps the fat image
descriptors from starving the stream-start pieces), the scalar ring
stays light (bias, chunk-1 weights, and later the output stores), and
N=256 warmup matmuls on a memset tile bridge the PE from dispatch-ready
to data-ready so the HAM clock-gate busy window runs into the stream.
"""

from contextlib import ExitStack

import ml_dtypes
import numpy as np

import concourse.bass as bass
import concourse.tile as tile
from concourse import bacc, mybir
from concourse.bass_utils import run_bass_kernel_spmd

N_CORES = 8
B, C_IN, H, W = 32, 128, 64, 64
C_OUT, K = 256, 3
B_LOC = B // N_CORES          # images per core
N_CHUNK = C_OUT // 128        # cout chunks of 128
ROWS_PER_MM = 8               # 8 rows x 64 cols = 512 moving elements
N_RG = H // ROWS_PER_MM       # row groups per image
HP, WP = H + 2, W + 2         # padded

_COMPILED = None


def _build():
    nc = bacc.Bacc("TRN2", target_bir_lowering=False, debug=False,
                   num_devices=N_CORES)

    inp = nc.dram_tensor("inp", [B_LOC, C_IN, HP, WP], mybir.dt.bfloat16,
                         kind="ExternalInput").ap()
    # 1-column-shifted copy (padded cols 1:65): gives the kw=1 taps a
    # 4-byte-aligned, fully contiguous [128, 8x64] window — without it
    # they pay ~+7ns/matmul for 2-byte-misaligned SBUF reads.
    inp_s = nc.dram_tensor("inp_s", [B_LOC, C_IN, HP, W], mybir.dt.bfloat16,
                           kind="ExternalInput").ap()
    wt = nc.dram_tensor("wt", [N_CHUNK, C_IN, K * K, 128], mybir.dt.bfloat16,
                        kind="ExternalInput").ap()
    bias2 = nc.dram_tensor("bias2", [128, N_CHUNK], mybir.dt.float32,
                           kind="ExternalInput").ap()
    out = nc.dram_tensor("out", [B_LOC, C_OUT, H, W], mybir.dt.float32,
                         kind="ExternalOutput").ap()

    with tile.TileContext(nc) as tc, ExitStack() as ctx:
        consts = ctx.enter_context(tc.tile_pool(name="consts", bufs=1))
        pads = ctx.enter_context(tc.tile_pool(name="pads", bufs=1))
        outs = ctx.enter_context(tc.tile_pool(name="outs", bufs=10))
        psums = ctx.enter_context(tc.tile_pool(name="psums", bufs=6,
                                               space="PSUM"))
        wps = ctx.enter_context(tc.tile_pool(name="wps", bufs=1,
                                             space="PSUM"))

        # The two HWDGE rings pace everything by FIFO order — no SWDGE, no
        # timing hacks. Ring contents are ordered by need-by time, with the
        # stream-start gate (w chunk 0 + image-0 band 0) in front sharing
        # the early HBM window with nothing. Weights are chunk-major so
        # each cout-chunk is one contiguous 2304B-per-channel piece.
        w_r = consts.tile([128, N_CHUNK, K * K, 128], mybir.dt.bfloat16,
                          tag="w_r")
        b_sb = consts.tile([128, N_CHUNK], mybir.dt.float32, tag="b_sb")
        nc.sync.dma_start(out=w_r[:, 0], in_=wt[0])
        nc.scalar.dma_start(out=b_sb[:], in_=bias2[:])

        # Bridge the gap between PE dispatch-ready (~8us) and data-ready
        # (~10.9us) with N=256 matmuls on a memset tile: fine enough grain
        # that the last one ends within ~0.2us of the gate, keeping the HAM
        # clock-gate busy window continuous into the real stream.
        warm = consts.tile([128, 512], mybir.dt.bfloat16, tag="warm")
        nc.vector.memset(warm[:], 0.0)
        wpsum = wps.tile([128, ROWS_PER_MM * W], mybir.dt.float32,
                         tag="wpsum")
        for i in range(12):
            nc.tensor.matmul(wpsum[:, 0:256], warm[:, 0:128],
                             warm[:, 0:256], start=True, stop=True)

        pimgs = [pads.tile([128, HP, WP], mybir.dt.bfloat16,
                           name=f"pimg{i}", tag=f"pimg{i}")
                 for i in range(B_LOC)]
        pimgs_s = [pads.tile([128, HP, W], mybir.dt.bfloat16,
                             name=f"pimgs{i}", tag=f"pimgs{i}")
                   for i in range(B_LOC)]

        # One need-ordered data ring: the sync ring carries image 0's row
        # bands (rowgroup r reads padded rows 8r..8r+9) and then images 1-3
        # whole. Ring FIFO guarantees the bands are never starved by the
        # fat image descriptors (SDMA engines round-robin between rings
        # per PACKET, so a concurrent ring with 8712B descriptors would
        # take ~4x the bandwidth of the 2112B band pieces). The scalar
        # ring stays light — bias + chunk-1 weights — so the stores it
        # carries later never queue behind bulk loads.
        nc.scalar.dma_start(out=w_r[:, 1], in_=wt[1])
        bounds = [0, 10, 18, 34, 50, HP]
        for s in range(len(bounds) - 1):
            lo, hi = bounds[s], bounds[s + 1]
            nc.sync.dma_start(out=pimgs[0][:, lo:hi, :],
                              in_=inp[0, :, lo:hi, :])
            nc.sync.dma_start(out=pimgs_s[0][:, lo:hi, :],
                              in_=inp_s[0, :, lo:hi, :])
        for b in range(1, B_LOC):
            nc.sync.dma_start(out=pimgs[b][:], in_=inp[b])
            nc.sync.dma_start(out=pimgs_s[b][:], in_=inp_s[b])

        # Chunk-outer: all 8 rowgroups of chunk 0 (~15.5us of matmuls) run
        # before the first use of the chunk-1 weights, which land mid-sweep.
        for b in range(B_LOC):
            pimg = pimgs[b]
            for c in range(N_CHUNK):
                for r in range(N_RG):
                    acc = psums.tile([128, ROWS_PER_MM * W], mybir.dt.float32,
                                     tag="acc")
                    y0 = r * ROWS_PER_MM
                    for t in range(K * K):
                        kh, kw = divmod(t, K)
                        if kw == 1:
                            rhs = pimgs_s[b][:, y0 + kh:y0 + kh
                                             + ROWS_PER_MM, :]
                        else:
                            rhs = pimg[:, y0 + kh:y0 + kh + ROWS_PER_MM,
                                       kw:kw + W]
                        nc.tensor.matmul(acc[:],
                                         w_r[:, c, t, :],
                                         rhs,
                                         start=(t == 0), stop=(t == K * K - 1))
                    o = outs.tile([128, ROWS_PER_MM, W], mybir.dt.float32,
                                  tag="o")
                    acc_hw = acc[:].rearrange("p (h w) -> p h w",
                                              h=ROWS_PER_MM)
                    last = (b == B_LOC - 1 and r == N_RG - 1
                            and c == N_CHUNK - 1)
                    if last:
                        # Pipeline the exposed tail: activate + store in
                        # half-rowgroup pieces on both HWDGE rings, so the
                        # first store transfer overlaps the second half's
                        # activation instead of waiting for all of it.
                        h2 = ROWS_PER_MM // 2
                        nc.scalar.activation(o[:, 0:h2], acc_hw[:, 0:h2],
                                             mybir.ActivationFunctionType.Relu,
                                             bias=b_sb[:, c:c + 1], scale=1.0)
                        nc.sync.dma_start(
                            out=out[b, c * 128:(c + 1) * 128,
                                    y0:y0 + h2, :],
                            in_=o[:, 0:h2])
                        nc.scalar.activation(o[:, h2:ROWS_PER_MM],
                                             acc_hw[:, h2:ROWS_PER_MM],
                                             mybir.ActivationFunctionType.Relu,
                                             bias=b_sb[:, c:c + 1], scale=1.0)
                        nc.scalar.dma_start(
                            out=out[b, c * 128:(c + 1) * 128,
                                    y0 + h2:y0 + ROWS_PER_MM, :],
                            in_=o[:, h2:ROWS_PER_MM])
                    else:
                        nc.scalar.activation(o[:], acc_hw,
                                             mybir.ActivationFunctionType.Relu,
                                             bias=b_sb[:, c:c + 1], scale=1.0)
                        nc.scalar.dma_start(
                            out=out[b, c * 128:(c + 1) * 128,
                                    y0:y0 + ROWS_PER_MM, :],
                            in_=o[:])

    nc.compile()
    return nc


def _get_compiled():
    global _COMPILED
    if _COMPILED is None:
        _COMPILED = _build()
    return _COMPILED


def _run(inp, weight, bias, trace=False):
    inp = np.asarray(inp, dtype=np.float32)
    weight = np.asarray(weight, dtype=np.float32)
    bias = np.asarray(bias, dtype=np.float32)

    # Zero-pad to 66x66 and cast to bf16 host-side; also build the
    # 1-column-shifted copy (padded cols 1:65) for the kw=1 taps.
    inp_p = np.zeros((B, C_IN, HP, WP), dtype=np.float32)
    inp_p[:, :, 1:H + 1, 1:W + 1] = inp
    inp_p = inp_p.astype(ml_dtypes.bfloat16)
    inp_s = np.ascontiguousarray(inp_p[:, :, :, 1:W + 1])

    # weight [C_OUT, C_IN*K*K] -> [N_CHUNK, C_IN, K*K, 128] (chunk-major
    # lhsT layout: wt[c, ci, t, m] = weight[c*128+m, ci*9+t])
    wt = np.ascontiguousarray(
        weight.reshape(N_CHUNK, 128, C_IN, K * K).transpose(0, 2, 3, 1)
    ).astype(ml_dtypes.bfloat16)
    # bias [C_OUT] -> [128, N_CHUNK]: bias2[p, c] = bias[c*128 + p]
    bias2 = np.ascontiguousarray(bias.reshape(N_CHUNK, 128).T)

    nc = _get_compiled()
    in_maps = [
        {"inp": inp_p[i * B_LOC:(i + 1) * B_LOC],
         "inp_s": inp_s[i * B_LOC:(i + 1) * B_LOC],
         "wt": wt, "bias2": bias2}
        for i in range(N_CORES)
    ]
    res = run_bass_kernel_spmd(nc, in_maps, list(range(N_CORES)), trace=trace)
    full = np.concatenate([res.results[i]["out"] for i in range(N_CORES)],
                          axis=0)
    return full, res


def kernel(inp, weight, bias):
    full, _ = _run(inp, weight, bias, trace=False)
    return full
